# revision 11
# baseline (speedup 1.0000x reference)
"""Trainium2 Bass kernel for nn_AutoCorr2D.

Computation (per sample):
  f   = conv3x3(x, w_ext, pad=1) + b_ext            # [CC=128, 64, 64]
  corr[c,i,j,k] = f[c,i,j] * fpad[c, i+u-2, j+v-2]  # 5x5 window products
  out[o,i,j]    = sum_{c,k} w_reg[o,c,k] * corr[c,i,j,k] + b_reg[o]

Sharding: data-parallel over batch B=8 across 8 NeuronCores (one sample per
core); conv weights replicated.

Per-core implementation:
  stage 1: implicit GEMM over (cin_tile, 3x3 tap): 18 accumulating f32r
           matmuls per 512-pixel chunk, reading shifted views of a
           zero-padded x buffer; bias folded into the PSUM->SBUF copy
           (ScalarE Identity), which writes BF16 features twice: fpad and
           fpad_odd (the same features shifted one flat element earlier, so
           odd-column-shift taps read 4B-aligned bf16 pairs on DVE).
  stage 2: product symmetry: P_{a,b}[y,x] = f[y,x]*f[y+a,x+b] serves both
           tap (a,b) and tap (-a,-b) via shifted reads, so only 13 of 25
           product maps exist (ScalarE Square for (0,0), VectorE bf16
           tensor_mul at 2x/cycle for the rest).  The regressor GEMM packs
           the M=64 matmuls two-at-a-time into the PE's column-tiled
           128x64 mode (tile_position (0,0)/(0,64)): the two array halves
           stream different taps concurrently into psum[0:64]/[64:128].
           A fused DVE op (lo + b_reg) + hi produces the output tile.
  float32r streams at full PE rate for N>=256; bf16 likewise.  The PE is
  pre-warmed with dummy matmuls so the HAM clock gate releases early.
"""

import numpy as np

from concourse import bacc, mybir, tile
from concourse.bass_utils import run_bass_kernel_spmd

B, CIN, H, W = 8, 256, 64, 64
CC, COUT = 128, 64
HW = H * W
NCORES = 8

NCHUNK = 8           # pixel chunks per image
CROWS = H // NCHUNK  # rows per chunk (8) -> N = 512 pixels
NPX = CROWS * W      # 512
NGRP = 4             # product-map groups (2 chunks each)
GROWS = 2 * CROWS    # 16

XP = W + 2           # xpad cols (pad=1)
XR = H + 2           # xpad rows
FP = W + 4           # fpad cols (pad=2)
FR = H + 4           # fpad rows
FTAIL = 72           # guard tail so shifted product reads stay in-bounds

# The 13 "upper half" taps; (a,b) also serves tap (-a,-b) via a shifted read.
SYM = [(0, 0), (0, 1), (0, 2),
       (1, -2), (1, -1), (1, 0), (1, 1), (1, 2),
       (2, -2), (2, -1), (2, 0), (2, 1), (2, 2)]

F32 = mybir.dt.float32
F32R = mybir.dt.float32r
BF16 = mybir.dt.bfloat16
U32 = mybir.dt.uint32
U16 = mybir.dt.uint16
AF = mybir.ActivationFunctionType
ALU = mybir.AluOpType


def build_body(nc, tc, x, wext, wreg, bext, breg, out):
    with (
        tc.tile_pool(name="const", bufs=1) as constp,
        tc.tile_pool(name="xpadp", bufs=1) as xpadp,
        tc.tile_pool(name="fpadp", bufs=1) as fpadp,
        tc.tile_pool(name="prodp", bufs=3) as prodp,
        tc.tile_pool(name="outp", bufs=2) as outp,
        tc.tile_pool(name="ps1", bufs=3, space="PSUM") as ps1,
        tc.tile_pool(name="ps2a", bufs=2, space="PSUM") as ps2a,
        tc.tile_pool(name="ps2b", bufs=2, space="PSUM") as ps2b,
        tc.tile_pool(name="warmp", bufs=1, space="PSUM") as warmp,
    ):
        # PE warm-up: dummy matmuls on a zeroed f32r scratch start immediately
        # and release the HAM clock gate (~3.4us of activity) before real
        # matmuls begin.  N=128 keeps each mm short so the last one ends
        # right as the first conv chunk is ready (28 x ~107ns cold ~ 3us).
        wsc_r = constp.tile([128, 128], F32R, name="wsc_r")
        nc.vector.memset(wsc_r.bitcast(U32), 0)
        wpsum = warmp.tile([128, 128], F32, name="wpsum")
        for i in range(28):
            nc.tensor.matmul(wpsum, wsc_r, wsc_r,
                             start=(i == 0), stop=(i == 27))

        # ---- input DMAs on the Sync HWDGE queue (first = earliest data);
        # weights go on the Scalar engine's separate HWDGE queue so the two
        # transfer streams don't serialize.  f32 staging -> engine cast to
        # f32r (SWDGE casting DMAs cost ~30us of Q7 descriptor-gen time).
        xpads = []
        for t in range(2):
            xp = xpadp.tile([128, XR * XP], BF16, name=f"xpad{t}",
                            tag=f"xpad{t}")
            xr = xp.rearrange("p (r c) -> p r c", c=XP)
            xri = xp.bitcast(U16).rearrange("p (r c) -> p r c", c=XP)
            nc.vector.memset(xri[:, 0, :], 0)
            nc.vector.memset(xri[:, XR - 1, :], 0)
            nc.vector.memset(xri[:, 1:XR - 1, 0], 0)
            nc.vector.memset(xri[:, 1:XR - 1, XP - 1], 0)
            xpads.append(xr)

        # Weight DMAs on the Scalar engine's HWDGE queue (parallel to the
        # x stream on Sync); wext split so the first 9 lhsT blocks (cin
        # tile 0) land early.
        # bf16 conv weights: dtype != fp32 with 128-wide weight loads turns
        # on the compiler's Fast Weight Load (2 bf16/cycle), hiding LDWEIGHTS
        # under the previous matmul's streaming.
        w_st = constp.tile([128, 18 * 128], F32, name="w_st")
        wext_sb = constp.tile([128, 18 * 128], BF16, name="wext_sb")
        WSPLIT = ((0, 3), (3, 9), (9, 13), (13, 18))
        for lo, hi in WSPLIT:
            nc.scalar.dma_start(out=w_st[:, lo * 128:hi * 128],
                                in_=wext[:, lo * 128:hi * 128])
        for lo, hi in WSPLIT:
            nc.scalar.activation(wext_sb[:, lo * 128:hi * 128],
                                 w_st[:, lo * 128:hi * 128], AF.Copy)

        # x bands are 1:1 with stage-1 chunks: band i carries exactly the
        # input rows chunk i reads (i*8-1 .. i*8+9, overlapping by 2), so
        # each chunk waits on one small just-in-time DMA + cast.
        with tc.tile_pool(name="xstagep", bufs=3) as xstagep:
            xsts = []
            band_rows = []
            bext_sb = constp.tile([128, 1], F32, name="bext_sb")
            breg_sb = constp.tile([64, 1], F32, name="breg_sb")
            for band in range(NCHUNK):
                ra = max(band * CROWS - 1, 0)
                rb = min(band * CROWS + CROWS + 1, H)
                band_rows.append((ra, rb))
                pair = []
                for t in range(2):
                    xst = xstagep.tile([128, (rb - ra) * W], F32,
                                       name=f"xst{band}_{t}", tag="xst",
                                       padded_shape=[128, 10 * W])
                    src = x[t * 128:(t + 1) * 128, ra * W:rb * W]
                    nc.sync.dma_start(out=xst, in_=src)
                    pair.append(xst)
                xsts.append(pair)
                if band == 0:
                    nc.sync.dma_start(out=bext_sb, in_=bext)
                elif band == 4:
                    nc.sync.dma_start(out=breg_sb, in_=breg)
            # wreg reuses the wext staging tile (WAR orders it after casts);
            # its cast is emitted mid-stage-1 (ScalarE)
            nc.sync.dma_start(out=w_st[:, :25 * 64], in_=wreg)
            wreg_sb = constp.tile([128, 25 * 64], BF16, name="wreg_sb")

            def cast_band(band):
                # pad-scatter casts all on VectorE: keeps ScalarE's FIFO free
                # for the per-chunk bias-copies (no head-of-line blocking on
                # a band DMA), and VectorE is idle until products start
                ra, rb = band_rows[band]
                for t in range(2):
                    dst = xpads[t][:, 1 + ra:1 + rb, 1:1 + W]
                    stv = xsts[band][t].rearrange("p (r c) -> p r c", c=W)
                    nc.vector.tensor_copy(dst, stv)

            # ---- padded features (pad=2, bf16) + guard tail; fpad_odd is
            # fpad shifted one flat element earlier (odd[k] == fpad[k+1])
            # so taps with odd column shift read 4B-aligned bf16 pairs ----
            fpad = fpadp.tile([128, FR * FP + FTAIL], BF16, name="fpad")
            fodd = fpadp.tile([128, FR * FP + FTAIL], BF16, name="fodd")
            fr = fpad[:, :FR * FP].rearrange("p (r c) -> p r c", c=FP)
            fo = fodd[:, :FR * FP].rearrange("p (r c) -> p r c", c=FP)
            fpi = fpad.bitcast(U16)
            fri = fpi[:, :FR * FP].rearrange("p (r c) -> p r c", c=FP)
            foi_full = fodd.bitcast(U16)
            foi = foi_full[:, :FR * FP].rearrange("p (r c) -> p r c", c=FP)

            prodtiles = [[None] * len(SYM) for _ in range(NGRP)]

            def make_square(g):
                # tap (0,0) product map on ScalarE (Square), bf16
                base = (g * GROWS + 2) * FP
                pt = prodp.tile([128, GROWS * FP], BF16, name=f"sq{g}",
                                tag="prod0", bufs=4)
                nc.scalar.activation(pt, fpad[:, base:base + GROWS * FP],
                                     AF.Square)
                prodtiles[g][0] = pt

            # ---- stage 1: f = conv3x3(x) + b_ext ----
            # band casts interleave 1:1 with chunks so ScalarE's FIFO
            # reaches each chunk's bias-copy promptly; fpad border memsets
            # are emitted after band 0's casts so they don't delay chunk 0
            # in VectorE's FIFO (products don't need them until stage 2)
            for i in range(NCHUNK):
                cast_band(i)
                if i == 0:
                    # fpad borders: rows 0-1, bottom rows + tail, col pads
                    nc.vector.memset(fpi[:, 0:2 * FP], 0)
                    nc.vector.memset(fpi[:, (FR - 2) * FP:FR * FP + FTAIL], 0)
                    nc.vector.memset(fri[:, 2:FR - 2, 0:2], 0)
                    nc.vector.memset(fri[:, 2:FR - 2, FP - 2:FP], 0)
                    # fodd borders: the -1-shifted mirror of the above
                    nc.vector.memset(foi_full[:, 0:2 * FP], 0)
                    nc.vector.memset(
                        foi_full[:, (FR - 2) * FP - 1:FR * FP + FTAIL], 0)
                    nc.vector.memset(foi[:, 2:FR - 2, 0], 0)
                    nc.vector.memset(foi[:, 2:FR - 2, FP - 3:FP], 0)
                psum1 = ps1.tile([128, NPX], F32, name="psum1", tag="psum1")
                k = 0
                for t in range(2):
                    for du in range(3):
                        for dv in range(3):
                            rhs = xpads[t][:,
                                           i * CROWS + du:
                                           i * CROWS + du + CROWS,
                                           dv:dv + W]
                            blk = t * 9 + du * 3 + dv
                            lhsT = wext_sb[:, blk * 128:(blk + 1) * 128]
                            nc.tensor.matmul(psum1, lhsT, rhs,
                                             start=(k == 0), stop=(k == 17))
                            k += 1
                pv = psum1.rearrange("p (r c) -> p r c", c=W)
                dst = fr[:, i * CROWS + 2:i * CROWS + 2 + CROWS, 2:2 + W]
                nc.scalar.activation(dst, pv, AF.Identity,
                                     bias=bext_sb, scale=1.0)
                dsto = fo[:, i * CROWS + 2:i * CROWS + 2 + CROWS, 1:1 + W]
                nc.scalar.activation(dsto, pv, AF.Identity,
                                     bias=bext_sb, scale=1.0)
                # squares as soon as their 2-chunk group is complete; wreg
                # cast after chunk 2 (well before the first GEMM needs it)
                if i % 2 == 1:
                    make_square(i // 2)
                if i == 2:
                    nc.scalar.activation(wreg_sb, w_st[:, :25 * 64], AF.Copy)

            # ---- stage 2a: all product maps (VectorE, bf16 2x/cycle) ----
            # Emitted for all groups before any GEMM-finish op so the DVE
            # FIFO never head-of-line blocks products behind a finish that
            # waits on the PE.  bufs=3 means group 3 reuses group 0's
            # buffers (WAR: waits for GEMM chunk 1, done well before its
            # consumers chunks 6-7 start).
            for g in range(NGRP):
                for kk, (a, b) in enumerate(SYM):
                    if kk == 0:
                        continue
                    nrows = GROWS + a
                    base = (g * GROWS + 2 - a) * FP
                    pt = prodp.tile([128, nrows * FP], BF16,
                                    name=f"prod{g}_{kk}", tag=f"prod{kk}",
                                    bufs=3)
                    in0 = fpad[:, base:base + nrows * FP]
                    off = base + a * FP + b
                    if b % 2 == 0:
                        in1 = fpad[:, off:off + nrows * FP]
                    else:
                        in1 = fodd[:, off - 1:off - 1 + nrows * FP]
                    nc.vector.tensor_mul(pt, in0, in1)
                    prodtiles[g][kk] = pt

            # ---- stage 2b: regressor GEMM, column-tiled 128x64 pairs ----
            # 25 taps alternate between PE column tiles (0,0)->psum[0:64]
            # and (0,64)->psum[64:128]; the two array halves stream
            # different product views concurrently.  Finish fuses
            # (lo + b_reg) + hi on DVE straight out of PSUM.
            for i in range(NCHUNK):
                g = i // 2
                p8 = (i % 2) * CROWS
                ptiles = prodtiles[g]
                # separate banks per PE column tile: each half gets its own
                # accumulation group / has_written clear
                psum2l = ps2a.tile([128, NPX], F32, name="psum2l",
                                   tag="psum2l")
                psum2h = ps2b.tile([128, NPX], F32, name="psum2h",
                                   tag="psum2h")
                halves = [psum2l[0:64, :], psum2h[64:128, :]]
                mm = 0
                ntile = [0, 0]
                NT = [13, 12]
                for kk, (a, b) in enumerate(SYM):
                    pr = ptiles[kk].rearrange("p (r c) -> p r c", c=FP)
                    taps = ([(a, b)] if (a, b) == (0, 0)
                            else [(a, b), (-a, -b)])
                    for (p, q) in taps:
                        if kk == 0:
                            rhs = pr[:, p8:p8 + CROWS, 2:2 + W]
                        elif (p, q) == (a, b):
                            rhs = pr[:, p8 + a:p8 + a + CROWS, 2:2 + W]
                        else:
                            rhs = pr[:, p8:p8 + CROWS, 2 - b:2 - b + W]
                        tidx = (p + 2) * 5 + (q + 2)
                        lhsT = wreg_sb[:, tidx * 64:(tidx + 1) * 64]
                        half = mm % 2
                        cnt = ntile[half]
                        nc.tensor.matmul(halves[half], lhsT, rhs,
                                         start=(cnt == 0),
                                         stop=(cnt == NT[half] - 1),
                                         tile_position=(0, 64 * half))
                        ntile[half] += 1
                        mm += 1

                # ScalarE evacuates both halves promptly (releasing the PSUM
                # banks for the next chunks without waiting on the DVE FIFO,
                # which is deep in product maps); DVE later adds the two
                # SBUF tensors with no PSUM dependency.
                evl = outp.tile([COUT, NPX], F32, name="evl", tag="evl")
                nc.scalar.activation(evl, psum2l[0:64, :], AF.Copy)
                evh = outp.tile([COUT, NPX], F32, name="evh", tag="evh")
                nc.scalar.activation(evh, psum2h[64:128, :], AF.Identity,
                                     bias=breg_sb, scale=1.0)
                outt = outp.tile([COUT, NPX], F32, name="outsb",
                                 tag="outsb")
                nc.vector.tensor_add(outt, evl, evh)
                nc.sync.dma_start(out=out[:, i * NPX:(i + 1) * NPX],
                                  in_=outt)


def build_nc():
    nc = bacc.Bacc("TRN2", target_bir_lowering=False, debug=False,
                   num_devices=NCORES)
    x = nc.dram_tensor("x", [CIN, HW], F32, kind="ExternalInput").ap()
    wext = nc.dram_tensor("wext", [128, 18 * 128], F32,
                          kind="ExternalInput").ap()
    wreg = nc.dram_tensor("wreg", [128, 25 * 64], F32,
                          kind="ExternalInput").ap()
    bext = nc.dram_tensor("bext", [128, 1], F32, kind="ExternalInput").ap()
    breg = nc.dram_tensor("breg", [64, 1], F32, kind="ExternalInput").ap()
    out = nc.dram_tensor("out", [COUT, HW], F32, kind="ExternalOutput").ap()
    with tile.TileContext(nc) as tc:
        build_body(nc, tc, x, wext, wreg, bext, breg, out)
    nc.compile()
    return nc


def prep_in_maps(x, w_ext, b_ext, w_reg, b_reg):
    x = np.ascontiguousarray(np.asarray(x, dtype=np.float32))
    w_ext = np.asarray(w_ext, dtype=np.float32)
    w_reg = np.asarray(w_reg, dtype=np.float32)
    b_ext = np.asarray(b_ext, dtype=np.float32)
    b_reg = np.asarray(b_reg, dtype=np.float32)

    # lhsT layouts: wext [cin(128-part), (cintile,tap)*cc], wreg [cc, tap*cout]
    w1 = np.transpose(w_ext, (1, 2, 3, 0))          # [CIN, 3, 3, CC]
    wext_p = np.zeros((128, 18, 128), np.float32)
    for t in range(2):
        for du in range(3):
            for dv in range(3):
                wext_p[:, t * 9 + du * 3 + dv, :] = \
                    w1[t * 128:(t + 1) * 128, du, dv, :]
    wext_p = np.ascontiguousarray(wext_p.reshape(128, 18 * 128))
    w2 = np.transpose(w_reg, (1, 2, 3, 0))          # [CC, 5, 5, COUT]
    wreg_p = np.ascontiguousarray(w2.reshape(128, 25 * 64))
    bext_p = np.ascontiguousarray(b_ext.reshape(128, 1))
    breg_p = np.ascontiguousarray(b_reg.reshape(64, 1))

    return [{
        "x": np.ascontiguousarray(x[b].reshape(CIN, HW)),
        "wext": wext_p,
        "wreg": wreg_p,
        "bext": bext_p,
        "breg": breg_p,
    } for b in range(B)]


_NC_CACHE = None


def kernel(x, w_ext, b_ext, w_reg, b_reg):
    global _NC_CACHE
    if _NC_CACHE is None:
        _NC_CACHE = build_nc()
    nc = _NC_CACHE
    in_maps = prep_in_maps(x, w_ext, b_ext, w_reg, b_reg)
    res = run_bass_kernel_spmd(nc, in_maps, list(range(NCORES)))
    return np.stack([res.results[b]["out"].reshape(COUT, H, W)
                     for b in range(B)], axis=0)


# revision 13
# speedup vs baseline: 1.1794x; 1.1794x over previous
"""Trainium2 Bass kernel for nn_AutoCorr2D.

Computation (per sample):
  f   = conv3x3(x, w_ext, pad=1) + b_ext            # [CC=128, 64, 64]
  corr[c,i,j,k] = f[c,i,j] * fpad[c, i+u-2, j+v-2]  # 5x5 window products
  out[o,i,j]    = sum_{c,k} w_reg[o,c,k] * corr[c,i,j,k] + b_reg[o]

Sharding: data-parallel over batch B=8 across 8 NeuronCores (one sample per
core); conv weights replicated.

Per-core implementation:
  stage 1: implicit GEMM over (cin_tile, 3x3 tap): 18 accumulating f32r
           matmuls per 512-pixel chunk, reading shifted views of a
           zero-padded x buffer; bias folded into the PSUM->SBUF copy
           (ScalarE Identity), which writes BF16 features twice: fpad and
           fpad_odd (the same features shifted one flat element earlier, so
           odd-column-shift taps read 4B-aligned bf16 pairs on DVE).
  stage 2: product symmetry: P_{a,b}[y,x] = f[y,x]*f[y+a,x+b] serves both
           tap (a,b) and tap (-a,-b) via shifted reads, so only 13 of 25
           product maps exist (ScalarE Square for (0,0), VectorE bf16
           tensor_mul at 2x/cycle for the rest).  The regressor GEMM packs
           the M=64 matmuls two-at-a-time into the PE's column-tiled
           128x64 mode (tile_position (0,0)/(0,64)): the two array halves
           stream different taps concurrently into psum[0:64]/[64:128].
           A fused DVE op (lo + b_reg) + hi produces the output tile.
  float32r streams at full PE rate for N>=256; bf16 likewise.  The PE is
  pre-warmed with dummy matmuls so the HAM clock gate releases early.
"""

import numpy as np

from concourse import bacc, mybir, tile
from concourse.bass_utils import run_bass_kernel_spmd

B, CIN, H, W = 8, 256, 64, 64
CC, COUT = 128, 64
HW = H * W
NCORES = 8

NCHUNK = 8           # pixel chunks per image
CROWS = H // NCHUNK  # rows per chunk (8) -> N = 512 pixels
NPX = CROWS * W      # 512
NGRP = 4             # product-map groups (2 chunks each)
GROWS = 2 * CROWS    # 16

XP = W + 2           # xpad cols (pad=1)
XR = H + 2           # xpad rows
FP = W + 4           # fpad cols (pad=2)
FR = H + 4           # fpad rows
FTAIL = 72           # guard tail so shifted product reads stay in-bounds

# The 13 "upper half" taps; (a,b) also serves tap (-a,-b) via a shifted read.
SYM = [(0, 0), (0, 1), (0, 2),
       (1, -2), (1, -1), (1, 0), (1, 1), (1, 2),
       (2, -2), (2, -1), (2, 0), (2, 1), (2, 2)]

F32 = mybir.dt.float32
F32R = mybir.dt.float32r
BF16 = mybir.dt.bfloat16
U32 = mybir.dt.uint32
U16 = mybir.dt.uint16
AF = mybir.ActivationFunctionType
ALU = mybir.AluOpType


def build_body(nc, tc, x, wext, wreg, bext, breg, out):
    with (
        tc.tile_pool(name="const", bufs=1) as constp,
        tc.tile_pool(name="xpadp", bufs=1) as xpadp,
        tc.tile_pool(name="fpadp", bufs=1) as fpadp,
        tc.tile_pool(name="prodp", bufs=3) as prodp,
        tc.tile_pool(name="outp", bufs=2) as outp,
        tc.tile_pool(name="ps1", bufs=3, space="PSUM") as ps1,
        tc.tile_pool(name="ps2a", bufs=2, space="PSUM") as ps2a,
        tc.tile_pool(name="ps2b", bufs=2, space="PSUM") as ps2b,
        tc.tile_pool(name="warmp", bufs=1, space="PSUM") as warmp,
    ):
        # PE warm-up: dummy matmuls on a zeroed f32r scratch start immediately
        # and release the HAM clock gate (~3.4us of activity) before real
        # matmuls begin.
        wsc_r = constp.tile([128, NPX], F32R, name="wsc_r")
        nc.vector.memset(wsc_r.bitcast(U32), 0)
        wpsum = warmp.tile([128, NPX], F32, name="wpsum")
        for i in range(7):
            nc.tensor.matmul(wpsum, wsc_r[:, :128], wsc_r,
                             start=(i == 0), stop=(i == 6))

        # ---- input DMAs on the Sync HWDGE queue (first = earliest data);
        # weights go on the Scalar engine's separate HWDGE queue so the two
        # transfer streams don't serialize.  f32 staging -> engine cast to
        # f32r (SWDGE casting DMAs cost ~30us of Q7 descriptor-gen time).
        xpads = []
        for t in range(2):
            xp = xpadp.tile([128, XR * XP], BF16, name=f"xpad{t}",
                            tag=f"xpad{t}")
            xr = xp.rearrange("p (r c) -> p r c", c=XP)
            xri = xp.bitcast(U16).rearrange("p (r c) -> p r c", c=XP)
            nc.vector.memset(xri[:, 0, :], 0)
            nc.vector.memset(xri[:, XR - 1, :], 0)
            nc.vector.memset(xri[:, 1:XR - 1, 0], 0)
            nc.vector.memset(xri[:, 1:XR - 1, XP - 1], 0)
            xpads.append(xr)

        # Weight DMAs on the Scalar engine's HWDGE queue (parallel to the
        # x stream on Sync); wext split so the first 9 lhsT blocks (cin
        # tile 0) land early.
        # bf16 conv weights: dtype != fp32 with 128-wide weight loads turns
        # on the compiler's Fast Weight Load (2 bf16/cycle), hiding LDWEIGHTS
        # under the previous matmul's streaming.
        w_st = constp.tile([128, 18 * 128], F32, name="w_st")
        wext_sb = constp.tile([128, 18 * 128], BF16, name="wext_sb")
        WSPLIT = ((0, 3), (3, 9), (9, 13), (13, 18))
        for lo, hi in WSPLIT:
            nc.scalar.dma_start(out=w_st[:, lo * 128:hi * 128],
                                in_=wext[:, lo * 128:hi * 128])
        for lo, hi in WSPLIT:
            nc.scalar.activation(wext_sb[:, lo * 128:hi * 128],
                                 w_st[:, lo * 128:hi * 128], AF.Copy)

        # x bands are 1:1 with stage-1 chunks: band i carries exactly the
        # input rows chunk i reads (i*8-1 .. i*8+9, overlapping by 2), so
        # each chunk waits on one small just-in-time DMA + cast.
        # 6 staging buffers = 3 bands of DMA lookahead; with fewer, the
        # band DMAs WAR-throttle on the pad-scatter casts and the whole
        # x supply chain (DMA -> cast -> conv -> products) runs just-in-time,
        # stalling stage 1 by ~2us/chunk and letting the PE go HAM-cold.
        with tc.tile_pool(name="xstagep", bufs=6) as xstagep:
            xsts = []
            band_rows = []
            bext_sb = constp.tile([128, 1], F32, name="bext_sb")
            breg_sb = constp.tile([64, 1], F32, name="breg_sb")
            for band in range(NCHUNK):
                ra = max(band * CROWS - 1, 0)
                rb = min(band * CROWS + CROWS + 1, H)
                band_rows.append((ra, rb))
                pair = []
                for t in range(2):
                    xst = xstagep.tile([128, (rb - ra) * W], F32,
                                       name=f"xst{band}_{t}", tag="xst",
                                       padded_shape=[128, 10 * W])
                    src = x[t * 128:(t + 1) * 128, ra * W:rb * W]
                    nc.sync.dma_start(out=xst, in_=src)
                    pair.append(xst)
                xsts.append(pair)
                if band == 0:
                    nc.sync.dma_start(out=bext_sb, in_=bext)
                elif band == 4:
                    nc.sync.dma_start(out=breg_sb, in_=breg)
            # wreg reuses the wext staging tile (WAR orders it after casts);
            # its cast is emitted mid-stage-1 (ScalarE)
            nc.sync.dma_start(out=w_st[:, :25 * 64], in_=wreg)
            wreg_sb = constp.tile([128, 25 * 64], BF16, name="wreg_sb")

            def cast_band(band):
                # pad-scatter casts all on VectorE: keeps ScalarE's FIFO free
                # for the per-chunk bias-copies (no head-of-line blocking on
                # a band DMA), and VectorE is idle until products start
                ra, rb = band_rows[band]
                for t in range(2):
                    dst = xpads[t][:, 1 + ra:1 + rb, 1:1 + W]
                    stv = xsts[band][t].rearrange("p (r c) -> p r c", c=W)
                    nc.vector.tensor_copy(dst, stv)

            # ---- padded features (pad=2, bf16) + guard tail; fpad_odd is
            # fpad shifted one flat element earlier (odd[k] == fpad[k+1])
            # so taps with odd column shift read 4B-aligned bf16 pairs ----
            fpad = fpadp.tile([128, FR * FP + FTAIL], BF16, name="fpad")
            fodd = fpadp.tile([128, FR * FP + FTAIL], BF16, name="fodd")
            fr = fpad[:, :FR * FP].rearrange("p (r c) -> p r c", c=FP)
            fo = fodd[:, :FR * FP].rearrange("p (r c) -> p r c", c=FP)
            fpi = fpad.bitcast(U16)
            fri = fpi[:, :FR * FP].rearrange("p (r c) -> p r c", c=FP)
            foi_full = fodd.bitcast(U16)
            foi = foi_full[:, :FR * FP].rearrange("p (r c) -> p r c", c=FP)

            prodtiles = [[None] * len(SYM) for _ in range(NGRP)]

            def make_square(g):
                # tap (0,0) product map on ScalarE (Square), bf16
                base = (g * GROWS + 2) * FP
                pt = prodp.tile([128, GROWS * FP], BF16, name=f"sq{g}",
                                tag="prod0", bufs=4)
                nc.scalar.activation(pt, fpad[:, base:base + GROWS * FP],
                                     AF.Square)
                prodtiles[g][0] = pt

            # ---- stage 1: f = conv3x3(x) + b_ext ----
            # band casts interleave 1:1 with chunks so ScalarE's FIFO
            # reaches each chunk's bias-copy promptly; fpad border memsets
            # are emitted after band 0's casts so they don't delay chunk 0
            # in VectorE's FIFO (products don't need them until stage 2)
            for i in range(NCHUNK):
                cast_band(i)
                if i == 0:
                    # fpad borders: rows 0-1, bottom rows + tail, col pads
                    nc.vector.memset(fpi[:, 0:2 * FP], 0)
                    nc.vector.memset(fpi[:, (FR - 2) * FP:FR * FP + FTAIL], 0)
                    nc.vector.memset(fri[:, 2:FR - 2, 0:2], 0)
                    nc.vector.memset(fri[:, 2:FR - 2, FP - 2:FP], 0)
                    # fodd borders: the -1-shifted mirror of the above
                    nc.vector.memset(foi_full[:, 0:2 * FP], 0)
                    nc.vector.memset(
                        foi_full[:, (FR - 2) * FP - 1:FR * FP + FTAIL], 0)
                    nc.vector.memset(foi[:, 2:FR - 2, 0], 0)
                    nc.vector.memset(foi[:, 2:FR - 2, FP - 3:FP], 0)
                psum1 = ps1.tile([128, NPX], F32, name="psum1", tag="psum1")
                k = 0
                for t in range(2):
                    for du in range(3):
                        for dv in range(3):
                            rhs = xpads[t][:,
                                           i * CROWS + du:
                                           i * CROWS + du + CROWS,
                                           dv:dv + W]
                            blk = t * 9 + du * 3 + dv
                            lhsT = wext_sb[:, blk * 128:(blk + 1) * 128]
                            nc.tensor.matmul(psum1, lhsT, rhs,
                                             start=(k == 0), stop=(k == 17))
                            k += 1
                pv = psum1.rearrange("p (r c) -> p r c", c=W)
                dst = fr[:, i * CROWS + 2:i * CROWS + 2 + CROWS, 2:2 + W]
                nc.scalar.activation(dst, pv, AF.Identity,
                                     bias=bext_sb, scale=1.0)
                dsto = fo[:, i * CROWS + 2:i * CROWS + 2 + CROWS, 1:1 + W]
                nc.scalar.activation(dsto, pv, AF.Identity,
                                     bias=bext_sb, scale=1.0)
                # squares as soon as their 2-chunk group is complete; wreg
                # cast after chunk 2 (well before the first GEMM needs it)
                if i % 2 == 1:
                    make_square(i // 2)
                if i == 2:
                    nc.scalar.activation(wreg_sb, w_st[:, :25 * 64], AF.Copy)

            # ---- stage 2a: all product maps (VectorE, bf16 2x/cycle) ----
            # Emitted for all groups before any GEMM-finish op so the DVE
            # FIFO never head-of-line blocks products behind a finish that
            # waits on the PE.  bufs=3 means group 3 reuses group 0's
            # buffers (WAR: waits for GEMM chunk 1, done well before its
            # consumers chunks 6-7 start).
            for g in range(NGRP):
                for kk, (a, b) in enumerate(SYM):
                    if kk == 0:
                        continue
                    nrows = GROWS + a
                    base = (g * GROWS + 2 - a) * FP
                    pt = prodp.tile([128, nrows * FP], BF16,
                                    name=f"prod{g}_{kk}", tag=f"prod{kk}",
                                    bufs=3)
                    in0 = fpad[:, base:base + nrows * FP]
                    off = base + a * FP + b
                    if b % 2 == 0:
                        in1 = fpad[:, off:off + nrows * FP]
                    else:
                        in1 = fodd[:, off - 1:off - 1 + nrows * FP]
                    nc.vector.tensor_mul(pt, in0, in1)
                    prodtiles[g][kk] = pt

            # ---- stage 2b: regressor GEMM, column-tiled 128x64 pairs ----
            # 25 taps alternate between PE column tiles (0,0)->psum[0:64]
            # and (0,64)->psum[64:128]; the two array halves stream
            # different product views concurrently.  Finish fuses
            # (lo + b_reg) + hi on DVE straight out of PSUM.
            for i in range(NCHUNK):
                g = i // 2
                p8 = (i % 2) * CROWS
                ptiles = prodtiles[g]
                # separate banks per PE column tile: each half gets its own
                # accumulation group / has_written clear
                psum2l = ps2a.tile([128, NPX], F32, name="psum2l",
                                   tag="psum2l")
                psum2h = ps2b.tile([128, NPX], F32, name="psum2h",
                                   tag="psum2h")
                halves = [psum2l[0:64, :], psum2h[64:128, :]]
                mm = 0
                ntile = [0, 0]
                NT = [13, 12]
                for kk, (a, b) in enumerate(SYM):
                    pr = ptiles[kk].rearrange("p (r c) -> p r c", c=FP)
                    taps = ([(a, b)] if (a, b) == (0, 0)
                            else [(a, b), (-a, -b)])
                    for (p, q) in taps:
                        if kk == 0:
                            rhs = pr[:, p8:p8 + CROWS, 2:2 + W]
                        elif (p, q) == (a, b):
                            rhs = pr[:, p8 + a:p8 + a + CROWS, 2:2 + W]
                        else:
                            rhs = pr[:, p8:p8 + CROWS, 2 - b:2 - b + W]
                        tidx = (p + 2) * 5 + (q + 2)
                        lhsT = wreg_sb[:, tidx * 64:(tidx + 1) * 64]
                        half = mm % 2
                        cnt = ntile[half]
                        nc.tensor.matmul(halves[half], lhsT, rhs,
                                         start=(cnt == 0),
                                         stop=(cnt == NT[half] - 1),
                                         tile_position=(0, 64 * half))
                        ntile[half] += 1
                        mm += 1

                # ScalarE evacuates both halves promptly (releasing the PSUM
                # banks for the next chunks without waiting on the DVE FIFO,
                # which is deep in product maps); DVE later adds the two
                # SBUF tensors with no PSUM dependency.
                evl = outp.tile([COUT, NPX], F32, name="evl", tag="evl")
                nc.scalar.activation(evl, psum2l[0:64, :], AF.Copy)
                evh = outp.tile([COUT, NPX], F32, name="evh", tag="evh")
                nc.scalar.activation(evh, psum2h[64:128, :], AF.Identity,
                                     bias=breg_sb, scale=1.0)
                outt = outp.tile([COUT, NPX], F32, name="outsb",
                                 tag="outsb")
                nc.vector.tensor_add(outt, evl, evh)
                nc.sync.dma_start(out=out[:, i * NPX:(i + 1) * NPX],
                                  in_=outt)


def build_nc():
    nc = bacc.Bacc("TRN2", target_bir_lowering=False, debug=False,
                   num_devices=NCORES)
    x = nc.dram_tensor("x", [CIN, HW], F32, kind="ExternalInput").ap()
    wext = nc.dram_tensor("wext", [128, 18 * 128], F32,
                          kind="ExternalInput").ap()
    wreg = nc.dram_tensor("wreg", [128, 25 * 64], F32,
                          kind="ExternalInput").ap()
    bext = nc.dram_tensor("bext", [128, 1], F32, kind="ExternalInput").ap()
    breg = nc.dram_tensor("breg", [64, 1], F32, kind="ExternalInput").ap()
    out = nc.dram_tensor("out", [COUT, HW], F32, kind="ExternalOutput").ap()
    with tile.TileContext(nc) as tc:
        build_body(nc, tc, x, wext, wreg, bext, breg, out)
    nc.compile()
    return nc


def prep_in_maps(x, w_ext, b_ext, w_reg, b_reg):
    x = np.ascontiguousarray(np.asarray(x, dtype=np.float32))
    w_ext = np.asarray(w_ext, dtype=np.float32)
    w_reg = np.asarray(w_reg, dtype=np.float32)
    b_ext = np.asarray(b_ext, dtype=np.float32)
    b_reg = np.asarray(b_reg, dtype=np.float32)

    # lhsT layouts: wext [cin(128-part), (cintile,tap)*cc], wreg [cc, tap*cout]
    w1 = np.transpose(w_ext, (1, 2, 3, 0))          # [CIN, 3, 3, CC]
    wext_p = np.zeros((128, 18, 128), np.float32)
    for t in range(2):
        for du in range(3):
            for dv in range(3):
                wext_p[:, t * 9 + du * 3 + dv, :] = \
                    w1[t * 128:(t + 1) * 128, du, dv, :]
    wext_p = np.ascontiguousarray(wext_p.reshape(128, 18 * 128))
    w2 = np.transpose(w_reg, (1, 2, 3, 0))          # [CC, 5, 5, COUT]
    wreg_p = np.ascontiguousarray(w2.reshape(128, 25 * 64))
    bext_p = np.ascontiguousarray(b_ext.reshape(128, 1))
    breg_p = np.ascontiguousarray(b_reg.reshape(64, 1))

    return [{
        "x": np.ascontiguousarray(x[b].reshape(CIN, HW)),
        "wext": wext_p,
        "wreg": wreg_p,
        "bext": bext_p,
        "breg": breg_p,
    } for b in range(B)]


_NC_CACHE = None


def kernel(x, w_ext, b_ext, w_reg, b_reg):
    global _NC_CACHE
    if _NC_CACHE is None:
        _NC_CACHE = build_nc()
    nc = _NC_CACHE
    in_maps = prep_in_maps(x, w_ext, b_ext, w_reg, b_reg)
    res = run_bass_kernel_spmd(nc, in_maps, list(range(NCORES)))
    return np.stack([res.results[b]["out"].reshape(COUT, H, W)
                     for b in range(B)], axis=0)
